# revision 18
# baseline (speedup 1.0000x reference)
"""Compositional attention Trainium2 Bass kernel (V3: fused stream).

Sharding: 8 cores = 2 batches x 4 search-pairs.
Core c handles batch b=c//4 and searches (2*(c%4), 2*(c%4)+1); each core
produces a bf16 partial for its 128 rows of the S*D=512 concat dim
(host sums 4 partials per batch in fp32).

V3 notes (vs V2 baseline at ~358us):
  - Attention is a scalar-bound pipeline: per (search, query-block) the
    scores->exp->retrieve chain streams with double-buffered score psum;
    searches alternate so the two searches' 64-contract score matmuls
    land in different PE row groups and overlap.
  - Softmax denominators: flat contiguous DVE/GpSimd add tree + one
    ones-matmul partition reduce per block.
  - Composition epilogue is folded per-block: dot_r = (Wrk @ rqT) . E_r
    (saves the per-retrieval Wrk matmuls), per-query scalars bounce
    through DRAM as [128,4] tiles, sigmoid computed as 1/(1+exp(-x))
    so the ACT table never switches away from exp.
  - Wout projection streams during the tail of attention; output is
    bf16 (host accumulates in fp32).
"""

import sys

for _p in ("/opt/trn_rl_repo",):
    if _p not in sys.path:
        sys.path.insert(0, _p)

from contextlib import ExitStack

import ml_dtypes
import numpy as np

import concourse.bass as bass
import concourse.tile as tile
from concourse import bacc
from concourse import mybir
from concourse.bass import ts
from concourse.bass_utils import run_bass_kernel_spmd

B, N, DIM, S, R, D = 2, 2048, 1024, 8, 2, 64
NCORES = 8
SPC = 2          # searches per core
SD = SPC * D     # 128 (per-core slice of S*D)
RD = R * D       # 128
P = 128
IBL = 512        # query block
NIB = N // IBL   # 4
KC = DIM // P    # 8
NJT = N // P     # 16 key tiles
F32 = mybir.dt.float32
BF16 = mybir.dt.bfloat16
SCALE = float(D) ** -0.5
AF = mybir.ActivationFunctionType
ALU = mybir.AluOpType


def _emit(ctx: ExitStack, tc: tile.TileContext, io):
    nc = tc.nc
    xT, wq, wk, wr, wv, wrkT, wout, outp = io

    singles = ctx.enter_context(tc.tile_pool(name="singles", bufs=1))
    ones_b = singles.tile([P, 1], BF16)
    nc.vector.memset(ones_b, 1.0)
    ones_f64 = singles.tile([64, 1], F32)
    nc.vector.memset(ones_f64, 1.0)

    wq_sb = singles.tile([P, KC, SD], BF16)
    wk_sb = singles.tile([P, KC, SD], BF16)
    wr_sb = singles.tile([P, KC, SD], BF16)
    wv_sb = singles.tile([P, KC, RD], BF16)
    for dst, src in ((wq_sb, wq), (wk_sb, wk), (wr_sb, wr), (wv_sb, wv)):
        nc.sync.dma_start(out=dst, in_=src.rearrange("(kc p) m -> p kc m", p=P))
    wrkT_sb = singles.tile([P, D], BF16)   # rows 0:64 == rows 64:128 == Wrk^T
    nc.sync.dma_start(out=wrkT_sb, in_=wrkT)
    wout_sb = singles.tile([P, DIM], BF16)
    nc.sync.dma_start(out=wout_sb, in_=wout)

    # ACT exp-table warmup so the ~2.7us table load overlaps the prologue.
    warm = singles.tile([P, 4], F32)
    nc.vector.memset(warm, 0.0)
    warm2 = singles.tile([P, 4], F32)
    nc.scalar.activation(out=warm2, in_=warm, func=AF.Exp, scale=1.0)

    acts = ctx.enter_context(tc.tile_pool(name="acts", bufs=1))
    qT = acts.tile([P, N], BF16)
    kT = acts.tile([P, N], BF16)
    rqT = acts.tile([P, N], BF16)
    vnat = acts.tile([P, NJT, P], BF16)    # [key-in-tile, key-tile, rd]
    comp = acts.tile([P, N], BF16)         # composed output (both searches)
    E0 = [acts.tile([64, N], BF16, name=f"E0_{si}") for si in range(SPC)]
    E1 = [acts.tile([64, N], BF16, name=f"E1_{si}") for si in range(SPC)]
    rqw = [acts.tile([64, N], F32, name=f"rqw_{si}") for si in range(SPC)]

    xs = acts.tile([P, KC, N], BF16)
    for k in range(KC):
        nc.sync.dma_start(out=xs[:, k, :], in_=xT[ts(k, P), :])

    # ---- shared PSUM pools (8 banks total, all phases) ----
    scp = ctx.enter_context(
        tc.tile_pool(name="scp", bufs=2, space="PSUM"))   # [P,2,IBL] = 4 banks
    rtp = ctx.enter_context(
        tc.tile_pool(name="rtp", bufs=2, space="PSUM"))   # [P,IBL]   = 2 banks
    zdpp = ctx.enter_context(
        tc.tile_pool(name="zdpp", bufs=1, space="PSUM"))  # [33,IBL]  = 1 bank
    wpp = ctx.enter_context(
        tc.tile_pool(name="wpp", bufs=1, space="PSUM"))   # [P,IBL]   = 1 bank

    # ---- SBUF working pools ----
    etsp = ctx.enter_context(tc.tile_pool(name="etsp", bufs=2))
    trp1 = ctx.enter_context(tc.tile_pool(name="trp1", bufs=2))
    trp2 = ctx.enter_context(tc.tile_pool(name="trp2", bufs=2))
    trp3 = ctx.enter_context(tc.tile_pool(name="trp3", bufs=2))
    trp4 = ctx.enter_context(tc.tile_pool(name="trp4", bufs=2))
    dsp = ctx.enter_context(tc.tile_pool(name="dsp", bufs=3))
    m12p = ctx.enter_context(tc.tile_pool(name="m12p", bufs=2))
    dmp = ctx.enter_context(tc.tile_pool(name="dmp", bufs=2))
    rowp = ctx.enter_context(tc.tile_pool(name="rowp", bufs=1))
    bcp = ctx.enter_context(tc.tile_pool(name="bcp", bufs=2))
    ctp = ctx.enter_context(tc.tile_pool(name="ctp", bufs=2))
    obp = ctx.enter_context(tc.tile_pool(name="obp", bufs=3))

    def proj(wsb, dst, dst_dtype_f32=False):
        # dst[:, :] = (wsb chunkwise) @ x, written in two 1024-wide copies
        for half in range(2):
            ps = scp.tile([P, 2, IBL], F32, tag="sc", name=f"pj{half}")
            for c in range(2):
                for k in range(KC):
                    nc.tensor.matmul(
                        ps[:, c, :],
                        lhsT=wsb[:, k, :],
                        rhs=xs[:, k, ts(2 * half + c, IBL)],
                        start=(k == 0), stop=(k == KC - 1),
                    )
            nc.vector.tensor_copy(out=dst[:, ts(half, 2 * IBL)], in_=ps)

    proj(wk_sb, kT)
    proj(wq_sb, qT)

    def attn_block(ib, ets, rt):
        # scores -> exp -> retrieve for both searches, si-alternating
        for jg in range(NJT // 2):
            for si in range(SPC):
                lo = 64 * si
                sp = scp.tile([P, 2, IBL], F32, tag="sc", name=f"sc{si}")
                for h in range(2):
                    jt = 2 * jg + h
                    nc.tensor.matmul(
                        sp[:, h, :],
                        lhsT=kT[lo:lo + 64, ts(jt, P)],
                        rhs=qT[lo:lo + 64, ts(ib, IBL)],
                        start=True, stop=True,
                    )
                nc.scalar.activation(
                    out=ets[si][:, ts(jg, 2), :], in_=sp,
                    func=AF.Exp, scale=SCALE,
                )
                for h in range(2):
                    jt = 2 * jg + h
                    nc.tensor.matmul(
                        rt[si], lhsT=vnat[:, jt, :], rhs=ets[si][:, jt, :],
                        start=(jt == 0), stop=(jt == NJT - 1),
                        skip_group_check=True,
                    )

    def epilogue(ib, si, ets, rt):
        # softmax denominator partials: contiguous add tree (DVE+GpSimd)
        g1 = trp1.tile([P, 8, IBL], BF16, tag="g1")
        nc.vector.tensor_tensor(g1, ets[:, 0:8, :], ets[:, 8:16, :], ALU.add)
        g2 = trp2.tile([P, 4, IBL], BF16, tag="g2")
        nc.vector.tensor_tensor(g2, g1[:, 0:4, :], g1[:, 4:8, :], ALU.add)
        g3 = trp3.tile([P, 2, IBL], BF16, tag="g3")
        nc.gpsimd.tensor_tensor(g3, g2[:, 0:2, :], g2[:, 2:4, :], ALU.add)
        red = trp4.tile([P, IBL], BF16, tag="g4")
        nc.gpsimd.tensor_tensor(red, g3[:, 0, :], g3[:, 1, :], ALU.add)

        zdp = zdpp.tile([33, IBL], F32, tag="zd")
        nc.tensor.matmul(zdp[0:1, :], lhsT=ones_b, rhs=red,
                         start=True, stop=True)

        # stash retrieved halves (GPSIMD cannot read PSUM -> DVE)
        nc.vector.tensor_copy(out=E0[si][:, ts(ib, IBL)], in_=rt[0:64, :])
        nc.vector.tensor_copy(out=E1[si][:, ts(ib, IBL)], in_=rt[64:128, :])
        dsub = dsp.tile([64, IBL], F32, tag="ds")
        nc.gpsimd.tensor_tensor(dsub, E0[si][:, ts(ib, IBL)],
                                E1[si][:, ts(ib, IBL)], ALU.subtract)
        dmul = dmp.tile([64, IBL], F32, tag="dm")
        nc.gpsimd.tensor_tensor(dmul, rqw[si][:, ts(ib, IBL)], dsub, ALU.mult)
        nc.tensor.matmul(zdp[32:33, :], lhsT=ones_f64, rhs=dmul,
                         start=True, stop=True)

        # per-query scalars: row math on [1,512] lanes straight off PSUM
        # (TT base-partition mismatch is legal when one input is PSUM),
        # then GpSimd partition_broadcast from base-0 SBUF rows.
        invz = rowp.tile([1, IBL], F32, tag="invz")
        nc.vector.reciprocal(invz, zdp[0:1, :])
        xr = rowp.tile([1, IBL], F32, tag="xr")
        nc.vector.tensor_tensor(xr, zdp[32:33, :], invz, ALU.mult)
        er = rowp.tile([1, IBL], F32, tag="er")
        nc.scalar.activation(out=er, in_=xr, func=AF.Exp, scale=-SCALE)
        t1 = rowp.tile([1, IBL], F32, tag="t1")
        nc.vector.tensor_scalar_add(t1, er, 1.0)
        a0 = rowp.tile([1, IBL], F32, tag="a0")
        nc.vector.reciprocal(a0, t1)
        a0b = bcp.tile([64, IBL], F32, tag="a0b")
        nc.gpsimd.partition_broadcast(a0b, a0, channels=64)
        izb = bcp.tile([64, IBL], F32, tag="izb")
        nc.gpsimd.partition_broadcast(izb, invz, channels=64)

        # comp = (a0*(E0-E1) + E1) / Z   (all-SBUF operands -> GpSimd)
        m1 = m12p.tile([64, IBL], F32, tag="m1")
        nc.gpsimd.tensor_tensor(m1, a0b, dsub, ALU.mult)
        m2 = m12p.tile([64, IBL], F32, tag="m2")
        nc.vector.tensor_tensor(m2, m1, E1[si][:, ts(ib, IBL)], ALU.add)
        if si == 0:
            nc.gpsimd.tensor_tensor(comp[0:64, ts(ib, IBL)], m2, izb,
                                    ALU.mult)
        else:
            c1 = ctp.tile([64, IBL], BF16, tag="c1")
            nc.gpsimd.tensor_tensor(c1, m2, izb, ALU.mult)
            nc.gpsimd.dma_start(out=comp[64:128, ts(ib, IBL)], in_=c1)

    # ---- v in natural [keys, rd] layout, directly from x chunks ----
    for vg in range(4):
        vp = wpp.tile([P, 4, P], F32, tag="wo", name=f"vp{vg}")
        for c in range(4):
            jt = 4 * vg + c
            for k in range(KC):
                nc.tensor.matmul(
                    vp[:, c, :],
                    lhsT=xs[:, k, ts(jt, P)],
                    rhs=wv_sb[:, k, :],
                    start=(k == 0), stop=(k == KC - 1),
                )
        nc.vector.tensor_copy(out=vnat[:, ts(vg, 4), :], in_=vp)

    # ---- rq projection + rqw = Wrk @ rqT ----
    proj(wr_sb, rqT)
    for si in range(SPC):
        lo = 64 * si
        for c in range(NIB):
            rp = wpp.tile([64, IBL], F32, tag="wo", name=f"rp{si}{c}")
            nc.tensor.matmul(
                rp,
                lhsT=wrkT_sb[lo:lo + 64, :],
                rhs=rqT[lo:lo + 64, ts(c, IBL)],
                start=True, stop=True,
            )
            nc.vector.tensor_copy(out=rqw[si][:, ts(c, IBL)], in_=rp)

    for ib in range(NIB):
        ets = [etsp.tile([P, NJT, IBL], BF16, tag="ets", name=f"ets{ib}_{si}")
               for si in range(SPC)]
        rt = [rtp.tile([P, IBL], F32, tag="rt", name=f"rt{ib}_{si}")
              for si in range(SPC)]
        attn_block(ib, ets, rt)
        for si in range(SPC):
            epilogue(ib, si, ets[si], rt[si])

    # ---- output projection (bf16 partials; host sums in fp32) ----
    for nch in range(N // P):
        for h in range(DIM // IBL):
            pw = wpp.tile([P, IBL], F32, tag="wo", name=f"pw{nch}{h}")
            nc.tensor.matmul(pw, lhsT=comp[:, ts(nch, P)],
                             rhs=wout_sb[:, ts(h, IBL)], start=True, stop=True)
            ob = obp.tile([P, IBL], BF16, tag="ob")
            nc.vector.tensor_copy(out=ob, in_=pw)
            nc.sync.dma_start(out=outp[ts(nch, P), ts(h, IBL)], in_=ob)


def build_nc():
    nc = bacc.Bacc()
    xT = nc.declare_dram_parameter("xT", [DIM, N], BF16, isOutput=False)
    wq = nc.declare_dram_parameter("wq", [DIM, SD], BF16, isOutput=False)
    wk = nc.declare_dram_parameter("wk", [DIM, SD], BF16, isOutput=False)
    wr = nc.declare_dram_parameter("wr", [DIM, SD], BF16, isOutput=False)
    wv = nc.declare_dram_parameter("wv", [DIM, RD], BF16, isOutput=False)
    wrkT = nc.declare_dram_parameter("wrkT", [P, D], BF16, isOutput=False)
    wout = nc.declare_dram_parameter("wout", [SD, DIM], BF16, isOutput=False)
    outp = nc.declare_dram_parameter("outp", [N, DIM], BF16, isOutput=True)
    io = (xT[:], wq[:], wk[:], wr[:], wv[:], wrkT[:], wout[:], outp[:])
    with tile.TileContext(nc) as tc:
        with ExitStack() as ctx:
            _emit(ctx, tc, io)
    nc.compile()
    return nc


_CACHE = {}


def _get_nc():
    if "nc" not in _CACHE:
        _CACHE["nc"] = build_nc()
    return _CACHE["nc"]


def make_in_maps(x, Wsq, Wsk, Wrv, Wrq, Wrk, Wout):
    x = np.asarray(x, np.float32)
    bf = ml_dtypes.bfloat16
    wrkT_full = np.ascontiguousarray(
        np.concatenate([np.asarray(Wrk, np.float32).T] * 2, axis=0)
    ).astype(bf)
    in_maps = []
    for c in range(NCORES):
        b = c // 4
        s0 = 2 * (c % 4)
        sl = slice(s0 * D, (s0 + 2) * D)
        in_maps.append({
            "xT": np.ascontiguousarray(x[b].T).astype(bf),
            "wq": np.ascontiguousarray(np.asarray(Wsq, np.float32)[:, sl]).astype(bf),
            "wk": np.ascontiguousarray(np.asarray(Wsk, np.float32)[:, sl]).astype(bf),
            "wr": np.ascontiguousarray(np.asarray(Wrq, np.float32)[:, sl]).astype(bf),
            "wv": np.ascontiguousarray(np.asarray(Wrv, np.float32)).astype(bf),
            "wrkT": wrkT_full,
            "wout": np.ascontiguousarray(np.asarray(Wout, np.float32)[sl, :]).astype(bf),
        })
    return in_maps


def combine(results):
    out = np.zeros((B, N, DIM), np.float32)
    for c in range(NCORES):
        out[c // 4] += np.asarray(results[c]["outp"], np.float32)
    return out


def kernel(x, Wsq, Wsk, Wrv, Wrq, Wrk, Wout):
    nc = _get_nc()
    in_maps = make_in_maps(x, Wsq, Wsk, Wrv, Wrq, Wrk, Wout)
    res = run_bass_kernel_spmd(nc, in_maps, list(range(NCORES))).results
    return combine(res)


def _install_ntff_shim():
    """Provide antenv.axon_hooks in images that lack it, driving NTFF
    profiling via ctypes into the injected libaxon_pjrt.so."""
    import types
    import ctypes
    import contextlib

    try:
        from antenv.axon_hooks import get_axon_ntff_profile_hook  # noqa
        return
    except ImportError:
        pass
    so_path = "/opt/axon/libaxon_pjrt.so"
    lib = ctypes.CDLL(so_path)
    if not hasattr(lib, "axon_start_nrt_profile"):
        return
    lib.axon_start_nrt_profile.argtypes = [
        ctypes.POINTER(ctypes.c_int64), ctypes.c_size_t]
    lib.axon_start_nrt_profile.restype = ctypes.c_int64
    lib.axon_stop_nrt_profile.argtypes = [ctypes.c_char_p]
    lib.axon_stop_nrt_profile.restype = ctypes.c_int64

    @contextlib.contextmanager
    def _hook(output_dir, device_ids):
        import jax
        jax.devices()
        if device_ids:
            ids = (ctypes.c_int64 * len(device_ids))(*device_ids)
            rc = lib.axon_start_nrt_profile(ids, len(device_ids))
        else:
            rc = lib.axon_start_nrt_profile(None, 0)
        if rc != 0:
            raise RuntimeError(f"axon_start_nrt_profile rc={rc}")
        try:
            yield
        finally:
            n = lib.axon_stop_nrt_profile(str(output_dir).encode())
            print(f"profile: {n} file(s) written to {output_dir}")

    import antenv
    mod = types.ModuleType("antenv.axon_hooks")
    mod.get_axon_ntff_profile_hook = lambda: _hook
    mod.set_axon_ntff_profile_hook = lambda h: None
    sys.modules["antenv.axon_hooks"] = mod
    antenv.axon_hooks = mod


def run_traced(x, Wsq, Wsk, Wrv, Wrq, Wrk, Wout, **kw):
    _install_ntff_shim()
    nc = _get_nc()
    in_maps = make_in_maps(x, Wsq, Wsk, Wrv, Wrq, Wrk, Wout)
    br = run_bass_kernel_spmd(nc, in_maps, list(range(NCORES)), trace=True, **kw)
    return combine(br.results), br


# revision 23
# speedup vs baseline: 1.0310x; 1.0310x over previous
"""Compositional attention Trainium2 Bass kernel (V3: fused stream).

Sharding: 8 cores = 2 batches x 4 search-pairs.
Core c handles batch b=c//4 and searches (2*(c%4), 2*(c%4)+1); each core
produces a bf16 partial for its 128 rows of the S*D=512 concat dim
(host sums 4 partials per batch in fp32).

V3 notes (vs V2 baseline at ~358us):
  - Attention is a scalar-bound pipeline: per (search, query-block) the
    scores->exp->retrieve chain streams with double-buffered score psum;
    searches alternate so the two searches' 64-contract score matmuls
    land in different PE row groups and overlap.
  - Softmax denominators: flat contiguous DVE/GpSimd add tree + one
    ones-matmul partition reduce per block.
  - Composition epilogue is folded per-block: dot_r = (Wrk @ rqT) . E_r
    (saves the per-retrieval Wrk matmuls), per-query scalars bounce
    through DRAM as [128,4] tiles, sigmoid computed as 1/(1+exp(-x))
    so the ACT table never switches away from exp.
  - Wout projection streams during the tail of attention; output is
    bf16 (host accumulates in fp32).
"""

import sys

for _p in ("/opt/trn_rl_repo",):
    if _p not in sys.path:
        sys.path.insert(0, _p)

from contextlib import ExitStack

import ml_dtypes
import numpy as np

import concourse.bass as bass
import concourse.tile as tile
from concourse import bacc
from concourse import mybir
from concourse.bass import ts
from concourse.bass_utils import run_bass_kernel_spmd

B, N, DIM, S, R, D = 2, 2048, 1024, 8, 2, 64
NCORES = 8
SPC = 2          # searches per core
SD = SPC * D     # 128 (per-core slice of S*D)
RD = R * D       # 128
P = 128
IBL = 512        # query block
NIB = N // IBL   # 4
KC = DIM // P    # 8
NJT = N // P     # 16 key tiles
F32 = mybir.dt.float32
BF16 = mybir.dt.bfloat16
SCALE = float(D) ** -0.5
AF = mybir.ActivationFunctionType
ALU = mybir.AluOpType


def _emit(ctx: ExitStack, tc: tile.TileContext, io):
    nc = tc.nc
    xT, wq, wk, wr, wv, wrkT, wout, outp = io

    singles = ctx.enter_context(tc.tile_pool(name="singles", bufs=1))
    ones_b = singles.tile([P, 1], BF16)
    nc.vector.memset(ones_b, 1.0)
    ones_f64 = singles.tile([64, 1], F32)
    nc.vector.memset(ones_f64, 1.0)

    wq_sb = singles.tile([P, KC, SD], BF16)
    wk_sb = singles.tile([P, KC, SD], BF16)
    wr_sb = singles.tile([P, KC, SD], BF16)
    wv_sb = singles.tile([P, KC, RD], BF16)
    for dst, src in ((wq_sb, wq), (wk_sb, wk), (wr_sb, wr), (wv_sb, wv)):
        nc.sync.dma_start(out=dst, in_=src.rearrange("(kc p) m -> p kc m", p=P))
    wrkT_sb = singles.tile([P, D], BF16)   # rows 0:64 == rows 64:128 == Wrk^T
    nc.sync.dma_start(out=wrkT_sb, in_=wrkT)
    wout_sb = singles.tile([P, DIM], BF16)
    nc.sync.dma_start(out=wout_sb, in_=wout)

    # ACT exp-table warmup so the ~2.7us table load overlaps the prologue.
    warm = singles.tile([P, 4], F32)
    nc.vector.memset(warm, 0.0)
    warm2 = singles.tile([P, 4], F32)
    nc.scalar.activation(out=warm2, in_=warm, func=AF.Exp, scale=1.0)

    acts = ctx.enter_context(tc.tile_pool(name="acts", bufs=1))
    qT = acts.tile([P, N], BF16)
    kT = acts.tile([P, N], BF16)
    rqT = acts.tile([P, N], BF16)
    vnat = acts.tile([P, NJT, P], BF16)    # [key-in-tile, key-tile, rd]
    comp = acts.tile([P, N], BF16)         # composed output (both searches)
    E0 = [acts.tile([64, N], BF16, name=f"E0_{si}") for si in range(SPC)]
    E1 = [acts.tile([64, N], BF16, name=f"E1_{si}") for si in range(SPC)]
    rqw = [acts.tile([64, N], F32, name=f"rqw_{si}") for si in range(SPC)]

    # ---- shared PSUM pools (8 banks total, all phases) ----
    scp = ctx.enter_context(
        tc.tile_pool(name="scp", bufs=2, space="PSUM"))   # [P,2,IBL] = 4 banks
    rtp = ctx.enter_context(
        tc.tile_pool(name="rtp", bufs=2, space="PSUM"))   # [P,IBL]   = 2 banks
    zdpp = ctx.enter_context(
        tc.tile_pool(name="zdpp", bufs=1, space="PSUM"))  # [33,IBL]  = 1 bank
    wpp = ctx.enter_context(
        tc.tile_pool(name="wpp", bufs=1, space="PSUM"))   # [P,IBL]   = 1 bank

    def proj(wsb, dst, xs):
        # dst[:, :] = (wsb chunkwise) @ x, written in two 1024-wide copies
        for half in range(2):
            ps = scp.tile([P, 2, IBL], F32, tag="sc", name=f"pj{half}")
            for c in range(2):
                for k in range(KC):
                    nc.tensor.matmul(
                        ps[:, c, :],
                        lhsT=wsb[:, k, :],
                        rhs=xs[:, k, ts(2 * half + c, IBL)],
                        start=(k == 0), stop=(k == KC - 1),
                    )
            nc.vector.tensor_copy(out=dst[:, ts(half, 2 * IBL)], in_=ps)

    # ---- prologue: x load, projections, vnat, rqw (xs freed after) ----
    with tc.tile_pool(name="xsp", bufs=1) as xsp:
        xs = xsp.tile([P, KC, N], BF16)
        for k in range(KC):
            nc.sync.dma_start(out=xs[:, k, :], in_=xT[ts(k, P), :])

        proj(wk_sb, kT, xs)
        proj(wq_sb, qT, xs)

        # v in natural [keys, rd] layout, directly from x chunks
        for vg in range(4):
            vp = wpp.tile([P, 4, P], F32, tag="wo", name=f"vp{vg}")
            for c in range(4):
                jt = 4 * vg + c
                for k in range(KC):
                    nc.tensor.matmul(
                        vp[:, c, :],
                        lhsT=xs[:, k, ts(jt, P)],
                        rhs=wv_sb[:, k, :],
                        start=(k == 0), stop=(k == KC - 1),
                    )
            nc.vector.tensor_copy(out=vnat[:, ts(vg, 4), :], in_=vp)

        proj(wr_sb, rqT, xs)
        for si in range(SPC):
            lo = 64 * si
            for c in range(NIB):
                rp = wpp.tile([64, IBL], F32, tag="wo", name=f"rp{si}{c}")
                nc.tensor.matmul(
                    rp,
                    lhsT=wrkT_sb[lo:lo + 64, :],
                    rhs=rqT[lo:lo + 64, ts(c, IBL)],
                    start=True, stop=True,
                )
                nc.vector.tensor_copy(out=rqw[si][:, ts(c, IBL)], in_=rp)

    # ---- SBUF working pools (allocated after xs is released) ----
    etsp = ctx.enter_context(tc.tile_pool(name="etsp", bufs=4))
    trp1 = ctx.enter_context(tc.tile_pool(name="trp1", bufs=2))
    trp2 = ctx.enter_context(tc.tile_pool(name="trp2", bufs=2))
    dsp = ctx.enter_context(tc.tile_pool(name="dsp", bufs=3))
    m12p = ctx.enter_context(tc.tile_pool(name="m12p", bufs=2))
    dmp = ctx.enter_context(tc.tile_pool(name="dmp", bufs=2))
    rowp = ctx.enter_context(tc.tile_pool(name="rowp", bufs=2))
    bcp = ctx.enter_context(tc.tile_pool(name="bcp", bufs=2))
    ctp = ctx.enter_context(tc.tile_pool(name="ctp", bufs=2))
    obp = ctx.enter_context(tc.tile_pool(name="obp", bufs=3))

    def attn_block(ib, ets, rt):
        # scores -> exp -> retrieve for both searches, si-alternating
        for jg in range(NJT // 2):
            for si in range(SPC):
                lo = 64 * si
                sp = scp.tile([P, 2, IBL], F32, tag="sc", name=f"sc{si}")
                for h in range(2):
                    jt = 2 * jg + h
                    nc.tensor.matmul(
                        sp[:, h, :],
                        lhsT=kT[lo:lo + 64, ts(jt, P)],
                        rhs=qT[lo:lo + 64, ts(ib, IBL)],
                        start=True, stop=True,
                    )
                nc.scalar.activation(
                    out=ets[si][:, ts(jg, 2), :], in_=sp,
                    func=AF.Exp, scale=SCALE,
                )
                for h in range(2):
                    jt = 2 * jg + h
                    nc.tensor.matmul(
                        rt[si], lhsT=vnat[:, jt, :], rhs=ets[si][:, jt, :],
                        start=(jt == 0), stop=(jt == NJT - 1),
                        skip_group_check=True,
                    )

    def epilogue(ib, si, ets, rt):
        # softmax denominator partials: 2-level contiguous DVE add tree,
        # then 4 accumulating ones-matmuls finish the reduction on PE
        g1 = trp1.tile([P, 8, IBL], BF16, tag="g1")
        nc.vector.tensor_tensor(g1, ets[:, 0:8, :], ets[:, 8:16, :], ALU.add)
        g2 = trp2.tile([P, 4, IBL], BF16, tag="g2")
        nc.vector.tensor_tensor(g2, g1[:, 0:4, :], g1[:, 4:8, :], ALU.add)

        zdp = zdpp.tile([33, IBL], F32, tag="zd")
        for q in range(4):
            nc.tensor.matmul(zdp[0:1, :], lhsT=ones_b, rhs=g2[:, q, :],
                             start=(q == 0), stop=(q == 3),
                             skip_group_check=True)

        # stash retrieved halves (GPSIMD cannot read PSUM -> DVE)
        nc.vector.tensor_copy(out=E0[si][:, ts(ib, IBL)], in_=rt[0:64, :])
        nc.vector.tensor_copy(out=E1[si][:, ts(ib, IBL)], in_=rt[64:128, :])
        dsub = dsp.tile([64, IBL], F32, tag="ds")
        nc.gpsimd.tensor_tensor(dsub, E0[si][:, ts(ib, IBL)],
                                E1[si][:, ts(ib, IBL)], ALU.subtract)
        dmul = dmp.tile([64, IBL], F32, tag="dm")
        nc.gpsimd.tensor_tensor(dmul, rqw[si][:, ts(ib, IBL)], dsub, ALU.mult)
        nc.tensor.matmul(zdp[32:33, :], lhsT=ones_f64, rhs=dmul,
                         start=True, stop=True)

        # per-query scalars: row math on [1,512] lanes straight off PSUM
        # (TT base-partition mismatch is legal when one input is PSUM),
        # then GpSimd partition_broadcast from base-0 SBUF rows.
        invz = rowp.tile([1, IBL], F32, tag="invz")
        nc.vector.reciprocal_approx_fast(out=invz, in_=zdp[0:1, :])
        xr = rowp.tile([1, IBL], F32, tag="xr")
        nc.vector.tensor_tensor(xr, zdp[32:33, :], invz, ALU.mult)
        er = rowp.tile([1, IBL], F32, tag="er")
        nc.scalar.activation(out=er, in_=xr, func=AF.Exp, scale=-SCALE)
        t1 = rowp.tile([1, IBL], F32, tag="t1")
        nc.vector.tensor_scalar_add(t1, er, 1.0)
        a0 = rowp.tile([1, IBL], F32, tag="a0")
        nc.vector.reciprocal_approx_fast(out=a0, in_=t1)
        a0b = bcp.tile([64, IBL], F32, tag="a0b")
        nc.gpsimd.partition_broadcast(a0b, a0, channels=64)
        izb = bcp.tile([64, IBL], F32, tag="izb")
        nc.gpsimd.partition_broadcast(izb, invz, channels=64)

        # comp = (a0*(E0-E1) + E1) / Z   (all-SBUF operands -> GpSimd)
        m1 = m12p.tile([64, IBL], F32, tag="m1")
        nc.gpsimd.tensor_tensor(m1, a0b, dsub, ALU.mult)
        m2 = m12p.tile([64, IBL], F32, tag="m2")
        nc.vector.tensor_tensor(m2, m1, E1[si][:, ts(ib, IBL)], ALU.add)
        if si == 0:
            nc.gpsimd.tensor_tensor(comp[0:64, ts(ib, IBL)], m2, izb,
                                    ALU.mult)
        else:
            c1 = ctp.tile([64, IBL], BF16, tag="c1")
            nc.gpsimd.tensor_tensor(c1, m2, izb, ALU.mult)
            nc.gpsimd.dma_start(out=comp[64:128, ts(ib, IBL)], in_=c1)

    for ib in range(NIB):
        ets = [etsp.tile([P, NJT, IBL], BF16, tag="ets", name=f"ets{ib}_{si}")
               for si in range(SPC)]
        rt = [rtp.tile([P, IBL], F32, tag="rt", name=f"rt{ib}_{si}")
              for si in range(SPC)]
        attn_block(ib, ets, rt)
        for si in range(SPC):
            epilogue(ib, si, ets[si], rt[si])

    # ---- output projection (bf16 partials; host sums in fp32) ----
    for nch in range(N // P):
        for h in range(DIM // IBL):
            pw = wpp.tile([P, IBL], F32, tag="wo", name=f"pw{nch}{h}")
            nc.tensor.matmul(pw, lhsT=comp[:, ts(nch, P)],
                             rhs=wout_sb[:, ts(h, IBL)], start=True, stop=True)
            ob = obp.tile([P, IBL], BF16, tag="ob")
            nc.vector.tensor_copy(out=ob, in_=pw)
            nc.sync.dma_start(out=outp[ts(nch, P), ts(h, IBL)], in_=ob)


def build_nc():
    nc = bacc.Bacc()
    xT = nc.declare_dram_parameter("xT", [DIM, N], BF16, isOutput=False)
    wq = nc.declare_dram_parameter("wq", [DIM, SD], BF16, isOutput=False)
    wk = nc.declare_dram_parameter("wk", [DIM, SD], BF16, isOutput=False)
    wr = nc.declare_dram_parameter("wr", [DIM, SD], BF16, isOutput=False)
    wv = nc.declare_dram_parameter("wv", [DIM, RD], BF16, isOutput=False)
    wrkT = nc.declare_dram_parameter("wrkT", [P, D], BF16, isOutput=False)
    wout = nc.declare_dram_parameter("wout", [SD, DIM], BF16, isOutput=False)
    outp = nc.declare_dram_parameter("outp", [N, DIM], BF16, isOutput=True)
    io = (xT[:], wq[:], wk[:], wr[:], wv[:], wrkT[:], wout[:], outp[:])
    with tile.TileContext(nc) as tc:
        with ExitStack() as ctx:
            _emit(ctx, tc, io)
    nc.compile()
    return nc


_CACHE = {}


def _get_nc():
    if "nc" not in _CACHE:
        _CACHE["nc"] = build_nc()
    return _CACHE["nc"]


def make_in_maps(x, Wsq, Wsk, Wrv, Wrq, Wrk, Wout):
    x = np.asarray(x, np.float32)
    bf = ml_dtypes.bfloat16
    wrkT_full = np.ascontiguousarray(
        np.concatenate([np.asarray(Wrk, np.float32).T] * 2, axis=0)
    ).astype(bf)
    in_maps = []
    for c in range(NCORES):
        b = c // 4
        s0 = 2 * (c % 4)
        sl = slice(s0 * D, (s0 + 2) * D)
        in_maps.append({
            "xT": np.ascontiguousarray(x[b].T).astype(bf),
            "wq": np.ascontiguousarray(np.asarray(Wsq, np.float32)[:, sl]).astype(bf),
            "wk": np.ascontiguousarray(np.asarray(Wsk, np.float32)[:, sl]).astype(bf),
            "wr": np.ascontiguousarray(np.asarray(Wrq, np.float32)[:, sl]).astype(bf),
            "wv": np.ascontiguousarray(np.asarray(Wrv, np.float32)).astype(bf),
            "wrkT": wrkT_full,
            "wout": np.ascontiguousarray(np.asarray(Wout, np.float32)[sl, :]).astype(bf),
        })
    return in_maps


def combine(results):
    out = np.zeros((B, N, DIM), np.float32)
    for c in range(NCORES):
        out[c // 4] += np.asarray(results[c]["outp"], np.float32)
    return out


def kernel(x, Wsq, Wsk, Wrv, Wrq, Wrk, Wout):
    nc = _get_nc()
    in_maps = make_in_maps(x, Wsq, Wsk, Wrv, Wrq, Wrk, Wout)
    res = run_bass_kernel_spmd(nc, in_maps, list(range(NCORES))).results
    return combine(res)


def _install_ntff_shim():
    """Provide antenv.axon_hooks in images that lack it, driving NTFF
    profiling via ctypes into the injected libaxon_pjrt.so."""
    import types
    import ctypes
    import contextlib

    try:
        from antenv.axon_hooks import get_axon_ntff_profile_hook  # noqa
        return
    except ImportError:
        pass
    so_path = "/opt/axon/libaxon_pjrt.so"
    lib = ctypes.CDLL(so_path)
    if not hasattr(lib, "axon_start_nrt_profile"):
        return
    lib.axon_start_nrt_profile.argtypes = [
        ctypes.POINTER(ctypes.c_int64), ctypes.c_size_t]
    lib.axon_start_nrt_profile.restype = ctypes.c_int64
    lib.axon_stop_nrt_profile.argtypes = [ctypes.c_char_p]
    lib.axon_stop_nrt_profile.restype = ctypes.c_int64

    @contextlib.contextmanager
    def _hook(output_dir, device_ids):
        import jax
        jax.devices()
        if device_ids:
            ids = (ctypes.c_int64 * len(device_ids))(*device_ids)
            rc = lib.axon_start_nrt_profile(ids, len(device_ids))
        else:
            rc = lib.axon_start_nrt_profile(None, 0)
        if rc != 0:
            raise RuntimeError(f"axon_start_nrt_profile rc={rc}")
        try:
            yield
        finally:
            n = lib.axon_stop_nrt_profile(str(output_dir).encode())
            print(f"profile: {n} file(s) written to {output_dir}")

    import antenv
    mod = types.ModuleType("antenv.axon_hooks")
    mod.get_axon_ntff_profile_hook = lambda: _hook
    mod.set_axon_ntff_profile_hook = lambda h: None
    sys.modules["antenv.axon_hooks"] = mod
    antenv.axon_hooks = mod


def run_traced(x, Wsq, Wsk, Wrv, Wrq, Wrk, Wout, **kw):
    _install_ntff_shim()
    nc = _get_nc()
    in_maps = make_in_maps(x, Wsq, Wsk, Wrv, Wrq, Wrk, Wout)
    br = run_bass_kernel_spmd(nc, in_maps, list(range(NCORES)), trace=True, **kw)
    return combine(br.results), br
